# revision 3
# baseline (speedup 1.0000x reference)
"""Chamfer loss (whole-image) on 8 Trainium2 NeuronCores.

Math (matches the reference):
  p: N=16384 render points (from img_render_points.reshape(-1, 2)).
  q: M=20736 grid points (y=10j, x=10i), i<192, j<108, m = i*108 + j.
  out = sum_n min_m ||p_n - q_m|| + sum_m min_n ||p_n - q_m||.

Strategy:
  * Row-min side (min over the grid) is separable because the grid is a
    Cartesian product: min_m d2 = min_i (px-10i)^2 + min_j (py-10j)^2.
    Each core computes it for its N/8 = 2048 points (brute force over the
    192 + 108 axis values; exact).
  * Col-min side is the dense part: the grid axis M (padded to 21504) is
    sharded across the 8 cores (2688 grid points = 21 partition-tiles per
    core); each core scans all N=16384 points. d2 is produced entirely on
    the TensorEngine with a K=11 matmul: d2[m,n] = qx*(-2px) + qy*(-2py)
    + q2 + p2, every fp32 factor expanded into bf16 hi/lo terms (bf16*bf16
    products are exact in fp32 and PSUM accumulates in fp32, so this is
    fp32-accurate while running at the 1 cycle/column bf16 matmul rate).
    The min over n runs on the VectorEngine with a custom fused DVE op
    (min(Src0,Src1) with a MIN accumulator) that ingests two streams per
    cycle: one directly from PSUM, one staged to SBUF by the ScalarEngine.
  * Host applies the final sqrt (monotonic, commutes with min) and sums.
"""

import numpy as np
import ml_dtypes

import concourse.bacc as bacc
import concourse.bass as bass
import concourse.mybir as mybir
import concourse.tile as tile
from concourse import bass_utils
from concourse._compat import get_trn_type

BF16 = ml_dtypes.bfloat16


def _ensure_ntff_hook():
    """This image's `antenv` lacks `axon_hooks`, which run_bass_kernel_spmd
    imports when trace=True. Install an equivalent shim backed by the ctypes
    NTFF driver from trn_agent_boot. Best-effort: failures leave tracing off."""
    try:
        import antenv  # noqa: F401
        try:
            from antenv.axon_hooks import get_axon_ntff_profile_hook  # noqa: F401
            return  # real module exists
        except ImportError:
            pass
        import sys
        import types

        from trn_agent_boot.trn_boot import _ntff_profile_via_ctypes

        mod = types.ModuleType("antenv.axon_hooks")
        _state = {"hook": None}
        mod.set_axon_ntff_profile_hook = lambda h: _state.__setitem__("hook", h)
        mod.get_axon_ntff_profile_hook = lambda: _state["hook"]
        sys.modules["antenv.axon_hooks"] = mod
        antenv.axon_hooks = mod
        so = "/opt/axon/libaxon_pjrt.so"
        import os

        if os.path.exists(so):
            mod.set_axon_ntff_profile_hook(_ntff_profile_via_ctypes(so))
    except Exception:
        pass


_ensure_ntff_hook()

# Problem constants (hardcoded: harness runs kernel.py standalone).
H, W, STRIDE = 1080, 1920, 10
NY = -(-H // STRIDE)        # 108 grid y-values
NX = -(-W // STRIDE)        # 192 grid x-values
M = NX * NY                 # 20736 grid points
N = 128 * 128 * 2 // 2      # 16384 render points
NCORES = 8
MT = 21                     # m-tiles (128 wide) per core
M_LOC = 128 * MT            # 2688 grid points per core (padded total 21504)
N_LOC = N // NCORES         # 2048 rowmin points per core
K = 11                      # bf16 split-precision contraction rows
QUAD = 2048                 # n-chunk handled per pipeline step (4 matmuls)
NQUAD = N // QUAD
FMAX = float(np.finfo(np.float32).max)

_minmin_op = None
_built = None


def _get_minmin_op():
    """Register (once) the fused DVE op: out = min(in0, in1),
    accum_out = min(s0, min over free dim of out)."""
    global _minmin_op
    if _minmin_op is not None:
        return _minmin_op
    from concourse import dve_ops as DO
    from concourse.dve_spec import Spec, Src0, Src1, C0, minn, lower
    from concourse.dve_uop import DveOpSpec

    name = "TENSOR_MIN_MIN_REDUCE_CHAMFER"
    for op in DO.OPS:
        if op.name == name:
            _minmin_op = op
            return op

    def _ref(in0, in1, c0, c1, c2):
        b = np.minimum(in0.astype(np.float32), in1.astype(np.float32))
        acc = np.minimum(
            np.asarray(c0, np.float32).reshape(-1, 1),
            b.reshape(b.shape[0], -1).min(axis=-1, keepdims=True),
        )
        return b, acc

    spec = Spec(body=minn(Src0, Src1), accum=minn, accum_init=C0, reference=_ref)
    opcode = DO._CUSTOM_DVE_ROW_BASE + len(DO.OPS)
    assert opcode < 0x20, "custom-DVE opcode rows exhausted"
    shas = {}
    for ver in ("v3", "v4"):
        s = DveOpSpec(name=name, opcode=opcode, uops=lower(spec, ver=ver), rd1_en=True)
        shas[ver] = s.sha(ver)
    op = DO.DveOp(name, spec, subdim=False, uops_sha=shas)
    DO.OPS.append(op)
    DO._SUB_OPCODE_FOR_NAME[name] = opcode
    DO.CUSTOM_DVE_SPECS[name] = spec
    _minmin_op = op
    return op


def _build():
    """Trace + compile the per-core Bass kernel once."""
    global _built
    if _built is not None:
        return _built
    op = _get_minmin_op()
    nc = bacc.Bacc(get_trn_type() or "TRN2", target_bir_lowering=False, debug=False)
    f32 = mybir.dt.float32
    bf16 = mybir.dt.bfloat16
    ACT = mybir.ActivationFunctionType
    ALU = mybir.AluOpType

    lhsT_d = nc.dram_tensor("lhsT", (K, M_LOC), bf16, kind="ExternalInput")
    rhs_d = nc.dram_tensor("rhs", (K, N), bf16, kind="ExternalInput")
    prow_d = nc.dram_tensor("prow", (128, 2, 16), f32, kind="ExternalInput")
    ysb_d = nc.dram_tensor("ysb", (128, NY), f32, kind="ExternalInput")
    xsb_d = nc.dram_tensor("xsb", (128, NX), f32, kind="ExternalInput")
    col_d = nc.dram_tensor("colmin_sq", (128, MT), f32, kind="ExternalOutput")
    row_d = nc.dram_tensor("rowmin_sq", (128, 16), f32, kind="ExternalOutput")

    with tile.TileContext(nc) as tc:
        with (
            tc.tile_pool(name="const", bufs=1) as cpool,
            tc.tile_pool(name="stage", bufs=3) as spool,
            tc.tile_pool(name="scr", bufs=1) as gpool,
            tc.tile_pool(name="accs", bufs=2) as apool,
            tc.tile_pool(name="rmin", bufs=2) as rpool,
            tc.tile_pool(name="psA", bufs=2, space=bass.MemorySpace.PSUM) as psA,
            tc.tile_pool(name="psB", bufs=2, space=bass.MemorySpace.PSUM) as psB,
        ):
            lhsT = cpool.tile([K, M_LOC], bf16)
            nc.sync.dma_start(lhsT[:], lhsT_d[:])
            rhs = cpool.tile([K, N], bf16)
            nc.sync.dma_start(rhs[:], rhs_d[:])
            prow = cpool.tile([128, 2, 16], f32)
            nc.sync.dma_start(prow[:], prow_d[:])
            ysb = cpool.tile([128, NY], f32)
            nc.sync.dma_start(ysb[:], ysb_d[:])
            xsb = cpool.tile([128, NX], f32)
            nc.sync.dma_start(xsb[:], xsb_d[:])
            colout = cpool.tile([128, MT], f32)
            rowout = cpool.tile([128, 16], f32)

            # ---- row-min side (separable; 2048 points on this core) ----
            negp = cpool.tile([128, 2, 16], f32)
            nc.vector.tensor_scalar_mul(negp[:], prow[:], -1.0)
            amin = cpool.tile([128, 16], f32)
            bmin = cpool.tile([128, 16], f32)
            for t in range(16):
                sqy = rpool.tile([128, NY], f32, tag="sq")
                nc.scalar.activation(sqy[:], ysb[:], ACT.Square, bias=negp[:, 0, t : t + 1])
                nc.vector.tensor_reduce(
                    out=amin[:, t : t + 1], in_=sqy[:], axis=mybir.AxisListType.X, op=ALU.min
                )
                sqx = rpool.tile([128, NX], f32, tag="sq")
                nc.scalar.activation(sqx[:], xsb[:], ACT.Square, bias=negp[:, 1, t : t + 1])
                nc.vector.tensor_reduce(
                    out=bmin[:, t : t + 1], in_=sqx[:], axis=mybir.AxisListType.X, op=ALU.min
                )
            nc.vector.scalar_tensor_tensor(
                out=rowout[:], in0=amin[:], scalar=0.0, in1=bmin[:],
                op0=ALU.add, op1=ALU.add,
            )
            nc.sync.dma_start(row_d[:], rowout[:])

            # ---- col-min side (dense K=11 matmul + fused min-reduce) ----
            for t in range(MT):
                wslice = lhsT[:, t * 128 : (t + 1) * 128]
                acc_prev = None
                for g in range(NQUAD):
                    n0 = g * QUAD
                    A = psA.tile([128, 1024], f32, tag="A")
                    B = psB.tile([128, 1024], f32, tag="B")
                    nc.tensor.matmul(A[:, 0:512], wslice, rhs[:, n0 : n0 + 512])
                    nc.tensor.matmul(A[:, 512:1024], wslice, rhs[:, n0 + 512 : n0 + 1024])
                    nc.tensor.matmul(B[:, 0:512], wslice, rhs[:, n0 + 1024 : n0 + 1536])
                    nc.tensor.matmul(B[:, 512:1024], wslice, rhs[:, n0 + 1536 : n0 + 2048])
                    S = spool.tile([128, 1024], f32, tag="S")
                    nc.scalar.activation(S[:], B[:], ACT.Copy)
                    garbage = gpool.tile([128, 1024], f32, tag="garb")
                    if g == NQUAD - 1:
                        acc_out = colout[:, t : t + 1]
                    else:
                        acc_out = apool.tile([128, 1], f32, tag="acc", name="acc")[:]
                    nc.vector._custom_dve(
                        op,
                        out=garbage[:],
                        in0=A[:],
                        in1=S[:],
                        s0=(FMAX if g == 0 else acc_prev),
                        accum_out=acc_out,
                    )
                    acc_prev = acc_out
            nc.sync.dma_start(col_d[:], colout[:])

    nc.compile()
    _built = nc
    return nc


def _split_bf16(v, n_terms):
    """Split float64 array into n_terms bf16 arrays with sum ~= v."""
    parts = []
    r = np.asarray(v, np.float64).copy()
    for _ in range(n_terms):
        p = r.astype(BF16)
        parts.append(p)
        r -= p.astype(np.float64)
    return parts


# Results of the most recent device run (exec_time_ns etc.), for test harnesses.
LAST_RUN = None


def kernel(img_render_points, img_ref):
    assert img_ref.shape == (H, W), f"unexpected img_ref shape {img_ref.shape}"
    p = np.asarray(img_render_points, np.float32).reshape(-1, 2).astype(np.float64)
    assert p.shape[0] == N

    pa = p[:, 0]  # pairs with grid y = 10j
    pb = p[:, 1]  # pairs with grid x = 10i

    # q-side (lhsT): padded grid, sharded across cores.
    M_PAD = M_LOC * NCORES
    m = np.arange(M_PAD)
    i = np.where(m < M, m // NY, 0)
    j = np.where(m < M, m % NY, 0)
    qb = (STRIDE * i).astype(np.float64)  # x
    qa = (STRIDE * j).astype(np.float64)  # y
    q2 = qa * qa + qb * qb
    qb_h, qb_l = _split_bf16(qb, 2)
    qa_h, qa_l = _split_bf16(qa, 2)
    q2_h, q2_m, q2_l = _split_bf16(q2, 3)
    ones_m = np.ones(M_PAD, BF16)
    lhsT_full = np.ascontiguousarray(
        np.stack([qb_h, qb_h, qb_l, qa_h, qa_h, qa_l, q2_h, q2_m, q2_l, ones_m, ones_m])
    )  # (K, M_PAD) bf16

    # p-side (rhs): shared by all cores.
    b_h, b_l = _split_bf16(-2.0 * pb, 2)
    a_h, a_l = _split_bf16(-2.0 * pa, 2)
    p2_h, p2_l = _split_bf16(pa * pa + pb * pb, 2)
    ones_n = np.ones(N, BF16)
    rhs = np.ascontiguousarray(
        np.stack([b_h, b_l, b_h, a_h, a_l, a_h, ones_n, ones_n, ones_n, p2_h, p2_l])
    )  # (K, N) bf16

    ysb = np.broadcast_to(
        (STRIDE * np.arange(NY)).astype(np.float32), (128, NY)
    ).copy()
    xsb = np.broadcast_to(
        (STRIDE * np.arange(NX)).astype(np.float32), (128, NX)
    ).copy()

    in_maps = []
    for c in range(NCORES):
        pa_c = pa[c * N_LOC : (c + 1) * N_LOC].astype(np.float32).reshape(128, 16)
        pb_c = pb[c * N_LOC : (c + 1) * N_LOC].astype(np.float32).reshape(128, 16)
        in_maps.append(
            {
                "lhsT": np.ascontiguousarray(lhsT_full[:, c * M_LOC : (c + 1) * M_LOC]),
                "rhs": rhs,
                "prow": np.ascontiguousarray(np.stack([pa_c, pb_c], axis=1)),
                "ysb": ysb,
                "xsb": xsb,
            }
        )

    nc = _build()
    global LAST_RUN
    LAST_RUN = bass_utils.run_bass_kernel_spmd(nc, in_maps, core_ids=list(range(NCORES)))

    colmins = np.concatenate(
        [r["colmin_sq"].T.reshape(-1) for r in LAST_RUN.results]
    )[:M]
    rowmins = np.concatenate([r["rowmin_sq"].reshape(-1) for r in LAST_RUN.results])
    total = (
        np.sqrt(np.maximum(colmins, 0.0).astype(np.float64)).sum()
        + np.sqrt(np.maximum(rowmins, 0.0).astype(np.float64)).sum()
    )
    return np.array(total, dtype=np.float32)
